# revision 26
# baseline (speedup 1.0000x reference)
"""Trainium2 Bass kernel for nn_MultiOmicsEmbedding (GNN message passing).

Strategy (8 cores, no collectives needed):
  - Destination-node-range sharding: core k owns nodes [6272k, 6272(k+1)).
    Every edge is routed to the core owning its dst node, so per-core
    partial aggregates are disjoint -> outputs just concatenate.
  - Per core, edges are bucketed by (subwindow = 128-dst-node group,
    src-half) into a fixed padded slot structure so all 8 cores run the
    SAME program (SPMD) with different data.
  - Node features are gathered with transposed dma_gather (fp16, 256B rows)
    producing [feature, edge] tiles directly (no PE transposes on inputs).
    The edge-attr tile and the b1-ones row are packed into spare rows of
    the gathered src tile so ONE matmul does x_src*Ws + ea*We + b1.
  - Message MLP runs in [hid, edge] orientation; one PE transpose station
    before aggregation; aggregation is a matmul with an on-device-built
    selection matrix (iota == dst_rel), accumulated per subwindow in PSUM.
  - Update linear (Wu) is applied after aggregation at node granularity.
  - Cell path is a plain fused linear+silu over the core's node slice.
"""

import sys

sys.path.insert(0, "/opt/trn_rl_repo")

from contextlib import ExitStack

import numpy as np

# --- problem constants (hardcoded; kernel.py must be self-contained) ---
N = 50000
E = 500000
ND = 64          # node feature dim
ED = 16          # edge feature dim
HID = 128
CELL_IN = 256
CELL_HID = 256

NCORES = 8
SW = 128                      # nodes per subwindow
SWS = 49                      # subwindows per core
CORE_NODES = SW * SWS         # 6272
NPAD = CORE_NODES * NCORES    # 50176
SPLIT = 25000                 # src table lo/hi split (int16 index limit)
CAP = 768                     # slot capacity per (subwindow, half) = 6 chunks
CH_PER_SW = CAP // 128        # 6
SEG_SLOTS = SWS * CAP         # 37632 slots per (modality, half) segment
ST = 512                      # supertile size (slots)
BATCH = 8192                  # gather batch (slots)

_cache = {}
TRACE = False
LAST_RES = None


def _batches():
    out = []
    o = 0
    while o < SEG_SLOTS:
        nb = min(BATCH, SEG_SLOTS - o)
        out.append((o, nb))
        o += nb
    return out


def _build_program():
    import concourse.bacc as bacc
    import concourse.mybir as mybir
    import concourse.tile as tile
    from concourse import library_config
    from concourse.masks import make_identity

    f16 = mybir.dt.float16
    f32 = mybir.dt.float32
    i16 = mybir.dt.int16
    AF = mybir.ActivationFunctionType

    nc = bacc.Bacc("TRN2", target_bir_lowering=False, debug=False)

    d = {}

    def di(name, shape, dt):
        d[name] = nc.dram_tensor(name, shape, dt, kind="ExternalInput").ap()

    def do(name, shape, dt):
        d[name] = nc.dram_tensor(name, shape, dt, kind="ExternalOutput").ap()

    for m in (0, 1):
        di(f"xlo{m}", [SPLIT, 128], f16)
        di(f"xhi{m}", [N - SPLIT, 128], f16)
        di(f"xdst{m}", [CORE_NODES, 128], f16)
        di(f"xsidx{m}", [128, 2 * SEG_SLOTS // 16], i16)
        di(f"xdidx{m}", [128, 2 * SEG_SLOTS // 16], i16)
        di(f"drl{m}", [128, 2 * SEG_SLOTS // 128], f32)
        di(f"eat{m}", [16, 2 * SEG_SLOTS], f16)
        di(f"w1d{m}", [128, 128], f16)
        di(f"w1c{m}", [128, 128], f16)
        di(f"w2{m}", [128, 128], f16)
        di(f"wu{m}", [128, 128], f16)
        di(f"b2{m}", [128, 1], f32)
        di(f"bu{m}", [128, 1], f32)
        do(f"y{m}t", [128, CORE_NODES], f32)
    for ki in (0, 1):
        for ko in (0, 1):
            di(f"wc{ki}{ko}", [128, 128], f16)
    di("bc0", [128, 1], f32)
    di("bc1", [128, 1], f32)
    di("cxt", [CELL_IN, CORE_NODES], f16)
    do("ct", [CELL_HID, CORE_NODES], f32)

    with tile.TileContext(nc) as tc, ExitStack() as ctx:
        const = ctx.enter_context(tc.tile_pool(name="const", bufs=1))
        gat = ctx.enter_context(tc.tile_pool(name="gat", bufs=3))
        hsb = ctx.enter_context(tc.tile_pool(name="hsb", bufs=3))
        ssb = ctx.enter_context(tc.tile_pool(name="ssb", bufs=3))
        accp = ctx.enter_context(tc.tile_pool(name="accp", bufs=1))
        psA = ctx.enter_context(tc.tile_pool(name="psA", bufs=2, space="PSUM"))
        psB = ctx.enter_context(tc.tile_pool(name="psB", bufs=2, space="PSUM"))
        psT = ctx.enter_context(tc.tile_pool(name="psT", bufs=2, space="PSUM"))
        psG = ctx.enter_context(tc.tile_pool(name="psG", bufs=2, space="PSUM"))

        nc.gpsimd.load_library(library_config.mlp)

        ident = const.tile([128, 128], f16)
        make_identity(nc, ident[:])
        iota16 = const.tile([128, 128], i16)
        nc.gpsimd.iota(iota16[:], pattern=[[1, 128]], base=0, channel_multiplier=0)
        iotaf = const.tile([128, 128], f16)
        nc.vector.tensor_copy(iotaf[:], iota16[:])

        w = {}
        for nm in list(d.keys()):
            if nm.startswith(("w1d", "w1c", "w2", "wu", "wc")):
                w[nm] = const.tile([128, 128], f16, tag=nm, name=nm)
                nc.sync.dma_start(w[nm][:], d[nm][:])
            elif nm.startswith(("b2", "bu", "bc")):
                w[nm] = const.tile([128, 1], f32, tag=nm, name=nm)
                nc.sync.dma_start(w[nm][:], d[nm][:])


        def emit_cell_tile(t):
            o = t * ST
            wid = min(ST, CORE_NODES - o)
            if wid <= 0:
                return
            cxa = gat.tile([128, ST], f16, tag="cxa")
            nc.sync.dma_start(cxa[:, :wid], d["cxt"][0:128, o:o + wid])
            cxb = gat.tile([128, ST], f16, tag="cxb")
            nc.sync.dma_start(cxb[:, :wid], d["cxt"][128:256, o:o + wid])
            for ko in (0, 1):
                cp = psB.tile([128, ST], f32, tag="h2p")
                nc.tensor.matmul(cp[:, :wid], w[f"wc0{ko}"][:], cxa[:, :wid],
                                 start=True, stop=False)
                nc.tensor.matmul(cp[:, :wid], w[f"wc1{ko}"][:], cxb[:, :wid],
                                 start=False, stop=True)
                ctt = hsb.tile([128, ST], f32, tag="ct")
                nc.scalar.activation(ctt[:, :wid], cp[:, :wid], AF.Silu,
                                     bias=w[f"bc{ko}"][:])
                nc.sync.dma_start(d["ct"][ko * 128:(ko + 1) * 128, o:o + wid],
                                  ctt[:, :wid])

        cell_next = [0]
        for m in (0, 1):
            acc = accp.tile([128, CORE_NODES], f32, tag="acc")
            nc.vector.memset(acc[:], 0.0)
            for half in (0, 1):
                xtab = d[f"xlo{m}"] if half == 0 else d[f"xhi{m}"]
                agg = None
                for (b0, nb) in _batches():
                    xsidx = ssb.tile([128, BATCH // 16], i16, tag="xsidx")
                    nc.sync.dma_start(
                        xsidx[:, : nb // 16],
                        d[f"xsidx{m}"][:, (half * SEG_SLOTS + b0) // 16:
                                       (half * SEG_SLOTS + b0 + nb) // 16],
                    )
                    xdidx = ssb.tile([128, BATCH // 16], i16, tag="xdidx")
                    nc.sync.dma_start(
                        xdidx[:, : nb // 16],
                        d[f"xdidx{m}"][:, (half * SEG_SLOTS + b0) // 16:
                                       (half * SEG_SLOTS + b0 + nb) // 16],
                    )
                    drl = ssb.tile([128, BATCH // 128], f32, tag="drl")
                    nc.sync.dma_start(
                        drl[:, : nb // 128],
                        d[f"drl{m}"][:, (half * SEG_SLOTS + b0) // 128:
                                     (half * SEG_SLOTS + b0 + nb) // 128],
                    )
                    xst = gat.tile([128, BATCH], f16, tag="xst")
                    nc.gpsimd.dma_gather(
                        xst[:, :nb].rearrange("p (o n) -> p o n", o=1),
                        xtab[:], xsidx[:, : nb // 16], nb, nb, 128,
                        transpose=True, single_packet=False,
                    )
                    nc.gpsimd.dma_start(
                        xst[64:80, :nb],
                        d[f"eat{m}"][:, half * SEG_SLOTS + b0:
                                     half * SEG_SLOTS + b0 + nb],
                    )
                    xdt = gat.tile([128, BATCH], f16, tag="xdt")
                    nc.gpsimd.dma_gather(
                        xdt[:, :nb].rearrange("p (o n) -> p o n", o=1),
                        d[f"xdst{m}"][:], xdidx[:, : nb // 16], nb, nb, 128,
                        transpose=True, single_packet=False,
                    )

                    for so in range(0, nb, ST):
                        S = min(ST, nb - so)
                        nch = S // 128
                        h1p = psA.tile([128, ST], f32, tag="h1p")
                        nc.tensor.matmul(h1p[:, :S], w[f"w1d{m}"][:],
                                         xdt[:, so:so + S], start=True, stop=False)
                        nc.tensor.matmul(h1p[:, :S], w[f"w1c{m}"][:],
                                         xst[:, so:so + S], start=False, stop=True)
                        h1s = hsb.tile([128, ST], f16, tag="h1s")
                        nc.scalar.activation(h1s[:, :S], h1p[:, :S], AF.Silu)

                        h2p = psB.tile([128, ST], f32, tag="h2p")
                        nc.tensor.matmul(h2p[:, :S], w[f"w2{m}"][:],
                                         h1s[:, :S], start=True, stop=True)
                        h2s = hsb.tile([128, ST], f16, tag="h2s")
                        nc.scalar.activation(h2s[:, :S], h2p[:, :S], AF.Silu,
                                             bias=w[f"b2{m}"][:])

                        trp = psT.tile([128, ST], f16, tag="trp")
                        for c in range(nch):
                            nc.tensor.transpose(trp[:, c * 128:(c + 1) * 128],
                                                h2s[:, c * 128:(c + 1) * 128],
                                                ident[:])
                        h3e = hsb.tile([128, ST], f16, tag="h3e")
                        nc.scalar.activation(h3e[:, :S], trp[:, :S], AF.Silu)

                        for c in range(nch):
                            g = (b0 + so) // 128 + c
                            sw_i = g // CH_PER_SW
                            slot = sw_i % 4
                            if agg is None:
                                agg = psG.tile([128, 512], f32, tag="agg")
                            selt = ssb.tile([128, 128], f16, tag="selt")
                            nc.vector.tensor_scalar(
                                selt[:], iotaf[:],
                                drl[:, (so // 128) + c:(so // 128) + c + 1],
                                None, op0=mybir.AluOpType.is_equal)
                            nc.tensor.matmul(
                                agg[:, slot * 128:(slot + 1) * 128],
                                h3e[:, c * 128:(c + 1) * 128], selt[:],
                                start=(g % CH_PER_SW == 0),
                                stop=(g % CH_PER_SW == CH_PER_SW - 1))
                            if g % CH_PER_SW == CH_PER_SW - 1 and \
                                    (slot == 3 or sw_i == SWS - 1):
                                wid = (slot + 1) * 128
                                k4 = sw_i // 4
                                nc.vector.tensor_tensor(
                                    acc[:, k4 * 512:k4 * 512 + wid],
                                    acc[:, k4 * 512:k4 * 512 + wid],
                                    agg[:, :wid], op=mybir.AluOpType.add)
                                agg = None
                    if cell_next[0] * ST < CORE_NODES:
                        emit_cell_tile(cell_next[0])
                        cell_next[0] += 1

            # ---- update linear: y_T = Wu^T @ acc + bu ----
            accf = accp.tile([128, CORE_NODES], f16, tag="accf")
            nc.vector.tensor_copy(accf[:], acc[:])
            o = 0
            while o < CORE_NODES:
                wid = min(512, CORE_NODES - o)
                yp = psA.tile([128, ST], f32, tag="h1p")
                nc.tensor.matmul(yp[:, :wid], w[f"wu{m}"][:],
                                 accf[:, o:o + wid], start=True, stop=True)
                yt = hsb.tile([128, ST], f32, tag="yt")
                nc.scalar.activation(yt[:, :wid], yp[:, :wid], AF.Identity,
                                     bias=w[f"bu{m}"][:])
                nc.sync.dma_start(d[f"y{m}t"][:, o:o + wid], yt[:, :wid])
                o += wid

        # cell path already interleaved into the edge phase above; emit
        # any remaining tiles here.
        while cell_next[0] * ST < CORE_NODES:
            emit_cell_tile(cell_next[0])
            cell_next[0] += 1

    nc.compile()
    return nc


def _prep_modality(x, ei, ea):
    """Build per-core staged arrays for one modality."""
    x = np.asarray(x, np.float32)
    ei = np.asarray(ei, np.int64)
    ea = np.asarray(ea, np.float32)
    src, dst = ei[0], ei[1]

    xpad = np.zeros((NPAD, 128), np.float16)
    xpad[:N, :ND] = x.astype(np.float16)
    xpad[:, 80] = 1.0

    core = dst // CORE_NODES
    loc = dst - core * CORE_NODES
    sw = loc >> 7
    dst_rel = (loc & 127).astype(np.float32)
    half = (src >= SPLIT).astype(np.int64)
    src_rel = (src - SPLIT * half).astype(np.int16)

    gid = (core * SWS + sw) * 2 + half
    order = np.argsort(gid, kind="stable")
    gs = gid[order]
    starts = np.searchsorted(gs, np.arange(NCORES * SWS * 2 + 1))
    cnt = np.diff(starts)
    assert cnt.max() <= CAP, f"subwindow overflow: {cnt.max()} > {CAP}"
    pos = np.arange(E) - starts[gs]

    flat = (core[order] * 2 + half[order]) * SEG_SLOTS + sw[order] * CAP + pos

    TOT = NCORES * 2 * SEG_SLOTS
    XS = np.zeros(TOT, np.int16)
    XS[flat] = src_rel[order]
    XD = np.zeros(TOT, np.int16)
    XD[flat] = loc[order].astype(np.int16)
    DR = np.full(TOT, 200.0, np.float32)
    DR[flat] = dst_rel[order]
    EA = np.zeros((TOT, ED), np.float16)
    EA[flat] = ea[order].astype(np.float16)

    XS = XS.reshape(NCORES, 2, SEG_SLOTS)
    XD = XD.reshape(NCORES, 2, SEG_SLOTS)
    DR = DR.reshape(NCORES, 2, SEG_SLOTS)
    EA = EA.reshape(NCORES, 2, SEG_SLOTS, ED)

    def stage_idx(a):  # [2, SEG] int16 -> [128, 2*SEG/16], idx block
        # replicated into all eight 16-partition stripes (one per Q7 core)
        out = np.empty((128, 2 * SEG_SLOTS // 16), np.int16)
        for h in (0, 1):
            out[:, h * (SEG_SLOTS // 16):(h + 1) * (SEG_SLOTS // 16)] = np.tile(
                a[h].reshape(SEG_SLOTS // 16, 16).T, (8, 1))
        return out

    percore = []
    for k in range(NCORES):
        drl = np.concatenate(
            [DR[k, h].reshape(SEG_SLOTS // 128, 128).T for h in (0, 1)], axis=1
        ).astype(np.float32)
        eat = np.concatenate([EA[k, h].T for h in (0, 1)], axis=1)
        percore.append({
            "xsidx": stage_idx(XS[k]),
            "xdidx": stage_idx(XD[k]),
            "drl": np.ascontiguousarray(drl),
            "eat": np.ascontiguousarray(eat),
            "xdst": np.ascontiguousarray(xpad[k * CORE_NODES:(k + 1) * CORE_NODES]),
        })
    shared = {
        "xlo": np.ascontiguousarray(xpad[:SPLIT]),
        "xhi": np.ascontiguousarray(xpad[SPLIT:N]),
    }
    return shared, percore


def _prep_weights(m, W1, b1, W2, b2, Wu, bu):
    W1 = np.asarray(W1, np.float32)
    w1d = np.zeros((128, 128), np.float16)
    w1d[:ND] = W1[:ND].astype(np.float16)
    w1c = np.zeros((128, 128), np.float16)
    w1c[:ND] = W1[ND:2 * ND].astype(np.float16)
    w1c[ND:ND + ED] = W1[2 * ND:2 * ND + ED].astype(np.float16)
    w1c[80] = np.asarray(b1, np.float16)
    return {
        f"w1d{m}": w1d,
        f"w1c{m}": w1c,
        f"w2{m}": np.asarray(W2, np.float16),
        f"wu{m}": np.asarray(Wu, np.float16),
        f"b2{m}": np.asarray(b2, np.float32).reshape(128, 1),
        f"bu{m}": np.asarray(bu, np.float32).reshape(128, 1),
    }


def kernel(x0, ei0, ea0, x1, ei1, ea1, cell_x,
           W1_0, b1_0, W2_0, b2_0, Wu_0, bu_0,
           W1_1, b1_1, W2_1, b2_1, Wu_1, bu_1,
           Wc, bc):
    from concourse.bass_utils import run_bass_kernel_spmd

    if "nc" not in _cache:
        _cache["nc"] = _build_program()
    nc = _cache["nc"]

    sh0, pc0 = _prep_modality(x0, ei0, ea0)
    sh1, pc1 = _prep_modality(x1, ei1, ea1)

    wmaps = {}
    wmaps.update(_prep_weights(0, W1_0, b1_0, W2_0, b2_0, Wu_0, bu_0))
    wmaps.update(_prep_weights(1, W1_1, b1_1, W2_1, b2_1, Wu_1, bu_1))
    Wc = np.asarray(Wc, np.float32)
    bc = np.asarray(bc, np.float32)
    for ki in (0, 1):
        for ko in (0, 1):
            wmaps[f"wc{ki}{ko}"] = np.ascontiguousarray(
                Wc[ki * 128:(ki + 1) * 128, ko * 128:(ko + 1) * 128]
            ).astype(np.float16)
    wmaps["bc0"] = bc[:128].reshape(128, 1)
    wmaps["bc1"] = bc[128:].reshape(128, 1)

    cxpad = np.zeros((NPAD, CELL_IN), np.float16)
    cxpad[:N] = np.asarray(cell_x, np.float32).astype(np.float16)

    in_maps = []
    for k in range(NCORES):
        im = {}
        for m, (sh, pc) in ((0, (sh0, pc0)), (1, (sh1, pc1))):
            im[f"xlo{m}"] = sh["xlo"]
            im[f"xhi{m}"] = sh["xhi"]
            im[f"xdst{m}"] = pc[k]["xdst"]
            im[f"xsidx{m}"] = pc[k]["xsidx"]
            im[f"xdidx{m}"] = pc[k]["xdidx"]
            im[f"drl{m}"] = pc[k]["drl"]
            im[f"eat{m}"] = pc[k]["eat"]
        im.update(wmaps)
        im["cxt"] = np.ascontiguousarray(
            cxpad[k * CORE_NODES:(k + 1) * CORE_NODES].T)
        in_maps.append(im)

    global LAST_RES
    res = run_bass_kernel_spmd(nc, in_maps, core_ids=list(range(NCORES)),
                               trace=TRACE)
    LAST_RES = res
    outs = res.results

    H = []
    for m in (0, 1):
        parts = []
        for k in range(NCORES):
            w_k = min(CORE_NODES, N - k * CORE_NODES)
            parts.append(outs[k][f"y{m}t"][:, :w_k].T)
        H.append(np.ascontiguousarray(np.concatenate(parts, 0), dtype=np.float32))
    cparts = []
    for k in range(NCORES):
        w_k = min(CORE_NODES, N - k * CORE_NODES)
        cparts.append(outs[k]["ct"][:, :w_k].T)
    C = np.ascontiguousarray(np.concatenate(cparts, 0), dtype=np.float32)
    return (H[0], H[1], C)
